# revision 2
# baseline (speedup 1.0000x reference)
"""CompresSAE topk-masking kernel for 8 Trainium2 NeuronCores.

Pipeline per core (data-parallel over batch, B_core rows):
  A) normalize x rows (scaled by 2^9), transpose -> fp16 hi tiles (xh16)
     plus fp8e4 residual/lo tiles (xc8 = [xl*2^15 ; x*2^5])
  B) encoder e*2^19 = one 12-matmul PSUM group per (chunk, block):
     6 fp16 matmuls (hi*hi) + 6 fp8e4 DoubleRow matmuls covering both
     cross terms (xl@wh + xh@wl), all at a common 2^19 scale.
     fused per-512-chunk screen: top-8 positive + top-8 negative values
     (+ chunk-local indices) per row -> 1024 candidates/row
  C) top-64-of-candidates per row via 8 rounds of (max8 + match_replace);
     masked candidate values = cand - zapped, signs+descale via signpat
  D) decoder out = e_masked @ Wd: rebuild per-chunk dense e_masked rows by
     gpsimd local_scatter, PE-transpose to [E,B] tiles, bf16 matmul,
     accumulated in PSUM over E; PE-transpose the output back to [B,D].
"""
import sys

for p in ("/opt/trn_rl_repo", "/root/.axon_site/_ro/trn_rl_repo"):
    if p not in sys.path:
        sys.path.insert(0, p)

import numpy as np

from concourse import bass_utils, tile, bacc
import concourse.mybir as mybir
from concourse.masks import make_identity

dt = mybir.dt
P = 128
D = 768
KD = D // P          # 6 contraction tiles
CHUNK = 512          # E-chunk width (= screen subchunk)
NSWEEP = 2           # decoder B-half sweeps (PSUM capacity)
TOPK = 64
DESCALE = 2.0 ** -19  # matmul group runs at 2^19 = (x*2^9)·(w*2^10)


def build(B_core: int, E: int, dbg: bool = False):
    nblk = B_core // P
    nchunk = E // CHUNK
    bps = nblk // NSWEEP          # blocks per decoder sweep
    ncand = 16 * nchunk           # candidates per row

    nc = bacc.Bacc(trn_type="TRN2", target_bir_lowering=False, debug=False)

    d_x = nc.dram_tensor("x", [B_core, D], dt.float32, kind="ExternalInput").ap()
    d_We = nc.dram_tensor("We", [D, E], dt.float32, kind="ExternalInput").ap()
    d_Wd = nc.dram_tensor("Wd", [E, D], dt.float32, kind="ExternalInput").ap()
    d_out = nc.dram_tensor("out", [B_core, D], dt.float32, kind="ExternalOutput").ap()

    with tile.TileContext(nc) as tc:
        with tc.tile_pool(name="consts", bufs=1) as consts, \
             tc.tile_pool(name="live", bufs=1) as live:
            ident_f = consts.tile([P, P], dt.float32)
            make_identity(nc, ident_f)
            ident_b = consts.tile([P, P], dt.bfloat16)
            make_identity(nc, ident_b)
            # sign+descale pattern over candidate slots: +s for pos-half, -s neg
            signpat = consts.tile([P, ncand // 16, 16], dt.float32)
            nc.vector.memset(signpat[:, :, 0:8], DESCALE)
            nc.vector.memset(signpat[:, :, 8:16], -DESCALE)

            # long-lived per-block arrays
            xh16 = [live.tile([P, KD, P], dt.float16, tag=f"xh{b}", name=f"xh{b}")
                    for b in range(nblk)]
            xc8 = [live.tile([P, 2 * KD, P], dt.float8e4, tag=f"xc{b}", name=f"xc{b}")
                   for b in range(nblk)]
            cand = [live.tile([P, ncand], dt.float32, tag=f"cand{b}", name=f"cand{b}") for b in range(nblk)]
            lidx = [live.tile([P, ncand], dt.uint16, tag=f"lidx{b}", name=f"lidx{b}") for b in range(nblk)]
            emcand = [live.tile([P, ncand], dt.bfloat16, tag=f"emc{b}", name=f"emc{b}") for b in range(nblk)]

            # ---------------- Phase A: normalize + transpose + split ------
            with tc.tile_pool(name="phA", bufs=2) as phA, \
                 tc.tile_pool(name="psA", bufs=4, space="PSUM") as psA:
                for b in range(nblk):
                    xb = phA.tile([P, D], dt.float32, tag="xb")
                    nc.gpsimd.dma_start(out=xb[:, :], in_=d_x[b * P:(b + 1) * P, :])
                    sq = phA.tile([P, D], dt.float32, tag="sq")
                    ss = phA.tile([P, 1], dt.float32, tag="ss")
                    nc.scalar.activation(sq[:, :], xb[:, :],
                                         mybir.ActivationFunctionType.Square,
                                         accum_out=ss[:, :])
                    # nrm = sqrt(ss)*2^-9 ; rn = 1/nrm = 2^9/|x|
                    nrm = phA.tile([P, 1], dt.float32, tag="nrm")
                    nc.scalar.activation(nrm[:, :], ss[:, :],
                                         mybir.ActivationFunctionType.Sqrt,
                                         scale=2.0 ** -18)
                    rn = phA.tile([P, 1], dt.float32, tag="rn")
                    nc.vector.reciprocal(rn[:, :], nrm[:, :])
                    xnb = phA.tile([P, D], dt.float32, tag="xnb")
                    nc.scalar.activation(xnb[:, :], xb[:, :],
                                         mybir.ActivationFunctionType.Copy,
                                         scale=rn[:, :])
                    # transpose 6 [128,128] tiles -> xnT (fp32, scaled 2^9)
                    xnT = phA.tile([P, KD, P], dt.float32, tag="xnT")
                    for g in range(2):      # two psum packs of 3 tiles
                        pk = psA.tile([P, 3 * P], dt.float32, tag="psA")
                        for j in range(3):
                            k = g * 3 + j
                            nc.tensor.transpose(pk[:, j * P:(j + 1) * P],
                                                xnb[:, k * P:(k + 1) * P],
                                                ident_f[:, :])
                        for j in range(3):
                            k = g * 3 + j
                            nc.scalar.copy(out=xnT[:, k, :],
                                           in_=pk[:, j * P:(j + 1) * P])
                    # fp16 hi + fp8 residual/lo splits
                    nc.vector.tensor_copy(out=xh16[b][:, :, :], in_=xnT[:, :, :])
                    xl32 = phA.tile([P, KD, P], dt.float32, tag="xl32")
                    nc.vector.tensor_sub(out=xl32[:, :, :], in0=xnT[:, :, :],
                                         in1=xh16[b][:, :, :])
                    # xl*2^9*2^6 = xl*2^15 ; x*2^9*2^-4 = x*2^5
                    nc.scalar.activation(xc8[b][:, 0:KD, :], xl32[:, :, :],
                                         mybir.ActivationFunctionType.Copy,
                                         scale=64.0)
                    nc.scalar.activation(xc8[b][:, KD:2 * KD, :], xnT[:, :, :],
                                         mybir.ActivationFunctionType.Copy,
                                         scale=0.0625)

            # ---------------- Phase B: encoder + fused screen -------------
            with tc.tile_pool(name="wstage", bufs=2) as wstage, \
                 tc.tile_pool(name="wtmp", bufs=1) as wtmp, \
                 tc.tile_pool(name="whl", bufs=2) as whl, \
                 tc.tile_pool(name="scr", bufs=4) as scr, \
                 tc.tile_pool(name="psB", bufs=1, space="PSUM") as psB:
                pse = [psB.tile([P, CHUNK], dt.float32, tag=f"pse{b}", name=f"pse{b}")
                       for b in range(nblk)]
                for c in range(nchunk):
                    wf = wstage.tile([P, KD, CHUNK], dt.float32, tag="wf")
                    nc.gpsimd.dma_start(
                        out=wf[:, :, :],
                        in_=d_We[:, c * CHUNK:(c + 1) * CHUNK].rearrange(
                            "(k p) n -> p k n", p=P))
                    # w*2^10 in fp32, then fp16 hi and fp8 lo splits
                    wfs = wtmp.tile([P, KD, CHUNK], dt.float32, tag="wfs")
                    nc.scalar.activation(wfs[:, :, :], wf[:, :, :],
                                         mybir.ActivationFunctionType.Copy,
                                         scale=1024.0)
                    wh16 = whl.tile([P, KD, CHUNK], dt.float16, tag="wh16")
                    nc.vector.tensor_copy(out=wh16[:, :, :], in_=wfs[:, :, :])
                    wl32 = wtmp.tile([P, KD, CHUNK], dt.float32, tag="wl32")
                    nc.vector.tensor_sub(out=wl32[:, :, :], in0=wfs[:, :, :],
                                         in1=wh16[:, :, :])
                    wc8 = whl.tile([P, 2 * KD, CHUNK], dt.float8e4, tag="wc8")
                    # w*2^4 ; wl*2^10*2^4 = wl*2^14
                    nc.scalar.activation(wc8[:, 0:KD, :], wf[:, :, :],
                                         mybir.ActivationFunctionType.Copy,
                                         scale=16.0)
                    nc.scalar.activation(wc8[:, KD:2 * KD, :], wl32[:, :, :],
                                         mybir.ActivationFunctionType.Copy,
                                         scale=16.0)
                    for b in range(nblk):
                        for k in range(KD):
                            nc.tensor.matmul(
                                pse[b][:, :], xh16[b][:, k, :], wh16[:, k, :],
                                start=(k == 0), stop=False)
                        for j in range(KD):
                            nc.tensor.matmul(
                                pse[b][:, :], xc8[b][:, 2 * j:2 * j + 2, :],
                                wc8[:, 2 * j:2 * j + 2, :],
                                start=False, stop=(j == KD - 1),
                                perf_mode=mybir.MatmulPerfMode.DoubleRow)
                        # negated eviction for the negative-side screen
                        en = scr.tile([P, CHUNK], dt.float32, tag="en")
                        nc.scalar.activation(en[:, :], pse[b][:, :],
                                             mybir.ActivationFunctionType.Copy,
                                             scale=-1.0)
                        # screens: top-8 of e (pos) and of -e (neg)
                        nc.vector.max(out=cand[b][:, 16 * c:16 * c + 8],
                                      in_=pse[b][:, :])
                        nc.vector.max_index(out=lidx[b][:, 16 * c:16 * c + 8],
                                            in_max=cand[b][:, 16 * c:16 * c + 8],
                                            in_values=pse[b][:, :])
                        nc.vector.max(out=cand[b][:, 16 * c + 8:16 * c + 16],
                                      in_=en[:, :])
                        nc.vector.max_index(
                            out=lidx[b][:, 16 * c + 8:16 * c + 16],
                            in_max=cand[b][:, 16 * c + 8:16 * c + 16],
                            in_values=en[:, :])

            # ---------------- Phase C helper: top-64 of candidates --------
            def emit_phaseC(phC, b):
                s1 = phC.tile([P, ncand], dt.float32, tag="s1", name=f"s1_{b}")
                s2 = phC.tile([P, ncand], dt.float32, tag="s2", name=f"s2_{b}")
                cur = cand[b]
                dst = s1
                for r in range(TOPK // 8):
                    v8 = phC.tile([P, 8], dt.float32, tag="v8", name=f"v8_{b}_{r}")
                    nc.vector.max(out=v8[:, :], in_=cur[:, :])
                    nc.vector.match_replace(out=dst[:, :],
                                            in_to_replace=v8[:, :],
                                            in_values=cur[:, :],
                                            imm_value=0.0)
                    cur, dst = dst, (s2 if dst is s1 else s1)
                dd = phC.tile([P, ncand], dt.float32, tag="dd", name=f"dd_{b}")
                nc.vector.tensor_sub(out=dd[:, :], in0=cand[b][:, :],
                                     in1=cur[:, :])
                nc.vector.tensor_mul(
                    out=emcand[b][:, :], in0=dd[:, :],
                    in1=signpat[:, :, :].rearrange("p a b -> p (a b)"))

            # ---------------- Phase D: decoder (with interleaved C) -------
            with tc.tile_pool(name="phC", bufs=2) as phC, \
                 tc.tile_pool(name="wdstage", bufs=2) as wdstage, \
                 tc.tile_pool(name="wdh", bufs=2) as wdhp, \
                 tc.tile_pool(name="emc", bufs=6) as emcp, \
                 tc.tile_pool(name="rhs", bufs=3) as rhsp, \
                 tc.tile_pool(name="tail", bufs=2) as tailp, \
                 tc.tile_pool(name="psD", bufs=1, space="PSUM") as psD, \
                 tc.tile_pool(name="psT", bufs=2, space="PSUM") as psT:
                EK = CHUNK // P   # 4 E-subtiles per chunk
                for sw in range(NSWEEP):
                    for bi in range(bps):
                        emit_phaseC(phC, sw * bps + bi)
                    pso = [psD.tile([P, bps * P], dt.float32, tag=f"pso{m}", name=f"pso{m}_{sw}")
                           for m in range(KD)]
                    for c in range(nchunk):
                        wdf = wdstage.tile([P, EK, D], dt.float32, tag="wdf")
                        nc.gpsimd.dma_start(
                            out=wdf[:, :, :],
                            in_=d_Wd[c * CHUNK:(c + 1) * CHUNK, :].rearrange(
                                "(k p) n -> p k n", p=P))
                        wdh = wdhp.tile([P, EK, D], dt.bfloat16, tag="wdh")
                        nc.vector.tensor_copy(out=wdh[:, :, :], in_=wdf[:, :, :])
                        # rebuild dense masked-e rows for this chunk + transpose
                        rhs = []
                        for es in range(EK):
                            pk = psT.tile([P, bps * P], dt.bfloat16, tag="psT", name=f"psT{sw}_{c}_{es}")
                            rhs.append((es, pk))
                        for bi in range(bps):
                            b = sw * bps + bi
                            em = emcp.tile([P, CHUNK], dt.bfloat16, tag="em")
                            nc.gpsimd.local_scatter(
                                em[:, :],
                                emcand[b][:, 16 * c:16 * c + 16],
                                lidx[b][:, 16 * c:16 * c + 16].bitcast(dt.int16),
                                channels=P, num_elems=CHUNK, num_idxs=16)
                            for (es, pk) in rhs:
                                nc.tensor.transpose(
                                    pk[:, bi * P:(bi + 1) * P],
                                    em[:, es * P:(es + 1) * P],
                                    ident_b[:, :])
                        rr = []
                        for (es, pk) in rhs:
                            rt = rhsp.tile([P, bps * P], dt.bfloat16,
                                           tag=f"rt{es}", name=f"rt{es}_{sw}_{c}")
                            nc.scalar.copy(out=rt[:, :], in_=pk[:, :])
                            rr.append(rt)
                        for m in range(KD):
                            for es in range(EK):
                                nc.tensor.matmul(
                                    pso[m][:, :],
                                    wdh[:, es, m * P:(m + 1) * P],
                                    rr[es][:, :],
                                    start=(c == 0 and es == 0),
                                    stop=(c == nchunk - 1 and es == EK - 1))
                    # tail: transpose out^T [D, bps*P] -> out rows
                    ot = [tailp.tile([P, bps * P], dt.float32, tag=f"ot{m}", name=f"ot{m}_{sw}")
                          for m in range(KD)]
                    for m in range(KD):
                        nc.scalar.copy(out=ot[m][:, :], in_=pso[m][:, :])
                    for bi in range(bps):
                        b = sw * bps + bi
                        ob = tailp.tile([P, D], dt.float32, tag="ob")
                        for g in range(2):
                            pk = psT.tile([P, 3 * P], dt.float32, tag="psT")
                            for j in range(3):
                                m = g * 3 + j
                                nc.tensor.transpose(
                                    pk[:, j * P:(j + 1) * P],
                                    ot[m][:, bi * P:(bi + 1) * P],
                                    ident_f[:, :])
                            nc.scalar.copy(out=ob[:, g * 3 * P:(g + 1) * 3 * P],
                                           in_=pk[:, :])
                        nc.gpsimd.dma_start(out=d_out[b * P:(b + 1) * P, :],
                                            in_=ob[:, :])

    nc.compile()
    return nc


_CACHE = {}


def _get(B_core, E):
    key = (B_core, E)
    if key not in _CACHE:
        _CACHE[key] = build(B_core, E)
    return _CACHE[key]


def kernel(x, encoder_w, encoder_b, decoder_w, k, n_cores=8):
    x = np.ascontiguousarray(np.asarray(x, dtype=np.float32))
    We = np.ascontiguousarray(np.asarray(encoder_w, dtype=np.float32))
    Wd = np.ascontiguousarray(np.asarray(decoder_w, dtype=np.float32))
    b = np.asarray(encoder_b)
    assert int(np.asarray(k)) == TOPK, f"kernel compiled for k={TOPK}"
    assert not np.any(b), "nonzero encoder_b not supported"
    B, Dd = x.shape
    E = We.shape[1]
    assert Dd == D and B % n_cores == 0
    B_core = B // n_cores

    nc = _get(B_core, E)
    in_maps = [{"x": x[i * B_core:(i + 1) * B_core], "We": We, "Wd": Wd}
               for i in range(n_cores)]
    res = bass_utils.run_bass_kernel_spmd(nc, in_maps,
                                          core_ids=list(range(n_cores)))
    return np.concatenate([res.results[i]["out"] for i in range(n_cores)],
                          axis=0)
